# revision 6
# baseline (speedup 1.0000x reference)
"""Trainium2 Bass kernel for causal self-attention with RoPE.

Shapes: x (2, 2048, 2048), 16 heads x 128 head_dim.
Sharding: 8 cores = 2 batch x 4 head-groups (4 heads per core).
Each core computes q/k/v projections for its heads, RoPE, causal-masked
softmax attention, and a partial output projection (its head columns of
wo); the host sums the 4 partials per batch element.

Layout strategy (per core):
  - all matmul operands in bf16 (halves DMA + SBUF; matmul rate is the
    same 1 row/cycle as fp32r, accumulation stays fp32 in PSUM).
  - q,k built in transposed layout (head_dim on partitions, t free) so
    RoPE and the score matmuls need no on-device transposes.  The host
    permutes wq/wk columns so RoPE's even/odd pairs become the two
    partition halves, and pre-scales wq by 1/sqrt(head_dim).
  - projections run on 512-wide query slices (half the matmul count of
    256-wide, so less exposed PE pipeline latency).
  - scores computed as s^T (keys x q) per 256-query group; softmax skips
    the max-subtraction (scores are O(1) by construction); row sums via
    a 128-wide ones matmul that lands already partition-broadcast, so
    normalization needs no gpsimd hop; o and l share one PSUM bank.
  - v computed directly in (t x e) layout by using x^T as the stationary
    operand, so the p@v matmul needs no transposes anywhere.
  - fully-masked key blocks are skipped (host inspects the mask);
    partially-masked blocks get a multiplicative exp-mask on the DVE.
  - the output projection for a query group is interleaved right after
    its last head finalizes, filling attention-phase PE bubbles instead
    of serializing at the end.
  - DMA is spread over three queues: weights wq on the scalar queue,
    wk/wv on the gpsimd queue, activations/tables/outputs on sync, so
    the first matmuls can start ~6us in.
"""

import sys
from contextlib import ExitStack

if "/opt/trn_rl_repo" not in sys.path:
    sys.path.insert(0, "/opt/trn_rl_repo")

import numpy as np
import ml_dtypes

import concourse.bacc as bacc
import concourse.mybir as mybir
import concourse.tile as tile
from concourse.bass_utils import run_bass_kernel_spmd

B, T, D, NH, HD = 2, 2048, 2048, 16, 128
HPC = 4              # heads per core
PAIR = 256           # queries per attention group
NPAIR = T // PAIR    # 8
NCHUNK = T // HD     # 16 key chunks of 128
PROJ = 512           # queries per projection slice
NSLICE = T // PROJ   # 4
BF16 = mybir.dt.bfloat16
F32 = mybir.dt.float32
NPBF16 = np.dtype(ml_dtypes.bfloat16)


def _mask_structure(mask):
    """Classify each (query-group, key-chunk) block of the additive mask.

    Returns (statuses, maskt): statuses[j] is a list of
    (chunk, mask_tile_index_or_minus1) for blocks that must be computed;
    maskt is the packed (128, nmask, 256) array of deduplicated
    transposed mask tiles for partially-masked blocks.
    """
    statuses = []
    tiles = {}
    tile_list = []
    for j in range(NPAIR):
        q = slice(j * PAIR, (j + 1) * PAIR)
        lst = []
        for c in range(NCHUNK):
            k = slice(c * HD, (c + 1) * HD)
            sub = mask[q, k]
            if np.all(sub <= -1e8):
                continue
            if np.all(sub == 0.0):
                lst.append((c, -1))
            else:
                key = sub.tobytes()
                mi = tiles.get(key)
                if mi is None:
                    mi = len(tile_list)
                    tiles[key] = mi
                    tile_list.append(np.ascontiguousarray(sub.T))
                lst.append((c, mi))
        assert lst, f"query group {j} has every key block masked"
        statuses.append(lst)
    nmask = max(1, len(tile_list))
    maskt = np.zeros((HD, nmask, PAIR), np.float32)
    for i, t in enumerate(tile_list):
        assert np.all(t <= 64.0), "additive mask too large for exp-mask trick"
        maskt[:, i, :] = np.exp(t)
    return statuses, maskt


def _build_program(statuses, nmask):
    nc = bacc.Bacc(None, target_bir_lowering=False)

    xt_d = nc.dram_tensor("xt", [D, T], BF16, kind="ExternalInput")
    wq_d = nc.dram_tensor("wqt", [D, HPC * HD], BF16, kind="ExternalInput")
    wk_d = nc.dram_tensor("wkt", [D, HPC * HD], BF16, kind="ExternalInput")
    wv_d = nc.dram_tensor("wvt", [D, HPC * HD], BF16, kind="ExternalInput")
    wo_d = nc.dram_tensor("wot", [HPC * HD, D], BF16, kind="ExternalInput")
    cs_d = nc.dram_tensor("cs", [HD, 2, T], F32, kind="ExternalInput")
    mk_d = nc.dram_tensor("maskt", [HD, nmask, PAIR], BF16, kind="ExternalInput")
    ones_d = nc.dram_tensor("ones_sq", [HD, HD], BF16, kind="ExternalInput")
    out_d = nc.dram_tensor("out", [T, D], F32, kind="ExternalOutput")

    xt_ap = xt_d.ap().rearrange("(k p) t -> p k t", p=HD)
    wq_ap = wq_d.ap().rearrange("(k p) e -> p k e", p=HD)
    wk_ap = wk_d.ap().rearrange("(k p) e -> p k e", p=HD)
    wv_ap = wv_d.ap().rearrange("(k p) e -> p k e", p=HD)
    wo_ap = wo_d.ap().rearrange("(h p) e -> p h e", p=HD)
    EXP = mybir.ActivationFunctionType.Exp

    with tile.TileContext(nc) as tc, ExitStack() as top:
        constp = top.enter_context(tc.tile_pool(name="const", bufs=1))
        ones_sb = constp.tile([HD, HD], BF16)
        nc.gpsimd.dma_start(ones_sb[:], ones_d[:])

        qkp = top.enter_context(tc.tile_pool(name="qkp", bufs=1))
        # q heads at [:, h, :], k heads at [:, 4+h, :]
        qk_sb = qkp.tile([HD, 2 * HPC, T], BF16)

        # wv + xt pools span the q/k pass (prefetch) and the v pass
        with ExitStack() as vph:
            wvp = vph.enter_context(tc.tile_pool(name="wvp", side="right", bufs=1))
            wv_sb = wvp.tile([HD, NCHUNK, HPC * HD], BF16)
            xtp = vph.enter_context(tc.tile_pool(name="xtp", side="right", bufs=2))

            # ---- combined q/k projection pass (+ fused RoPE) ----
            with ExitStack() as ph:
                wp = ph.enter_context(tc.tile_pool(name="wp", side="right", bufs=1))
                csp = ph.enter_context(tc.tile_pool(name="csp", side="right", bufs=2))
                ropep = ph.enter_context(tc.tile_pool(name="ropep", side="right", bufs=2))
                pps = ph.enter_context(tc.tile_pool(name="pps", bufs=6, space="PSUM"))
                wqk_sb = wp.tile([HD, 2, NCHUNK, HPC * HD], BF16)
                # wq then wk on the scalar HWDGE queue, split per k-chunk so
                # the first matmuls start as soon as chunk 0 lands; wv (needed
                # only at the v pass) as one big DMA on the gpsimd queue
                for k in range(NCHUNK):
                    nc.scalar.dma_start(wqk_sb[:, 0, k, :], wq_ap[:, k, :])
                for k in range(NCHUNK):
                    nc.scalar.dma_start(wqk_sb[:, 1, k, :], wk_ap[:, k, :])
                nc.gpsimd.dma_start(wv_sb[:], wv_ap[:])
                for ns in range(NSLICE):
                    tsl = slice(ns * PROJ, (ns + 1) * PROJ)
                    xt = xtp.tile([HD, NCHUNK, PROJ], BF16, tag="xt")
                    if ns == 0:
                        # per-chunk DMAs: subtile deps let the first matmul
                        # start after ~1.5us instead of waiting for the full
                        # slice transfer
                        for k in range(NCHUNK):
                            nc.sync.dma_start(xt[:, k, :], xt_ap[:, k, tsl])
                    else:
                        nc.sync.dma_start(xt[:], xt_ap[:, :, tsl])
                    cs_sl = csp.tile([HD, 2, PROJ], F32, tag="cs")
                    nc.sync.dma_start(cs_sl[:], cs_d[:, :, tsl])
                    for wsel in range(2):
                        for h in range(HPC):
                            ps = pps.tile([HD, PROJ], F32, tag="ps")
                            hs = slice(h * HD, (h + 1) * HD)
                            for k in range(NCHUNK):
                                nc.tensor.matmul(
                                    ps[:],
                                    wqk_sb[:, wsel, k, hs],
                                    xt[:, k, :],
                                    start=(k == 0),
                                    stop=(k == NCHUNK - 1),
                                )
                            # RoPE: dst = raw*C + swap(raw)*S.  The swap is
                            # materialized by two ScalarE half-copies; VectorE
                            # does two multiplies and one add per tile.
                            dst = qk_sb[:, wsel * HPC + h, tsl]
                            sw = ropep.tile([HD, PROJ], F32, tag="sw")
                            nc.scalar.copy(sw[0:64, :], ps[64:128, :])
                            nc.scalar.copy(sw[64:128, :], ps[0:64, :])
                            tb = ropep.tile([HD, PROJ], F32, tag="tb")
                            nc.vector.tensor_mul(dst, ps[:], cs_sl[:, 0, :])
                            nc.vector.tensor_mul(tb[:], sw[:], cs_sl[:, 1, :])
                            nc.vector.tensor_add(dst, dst, tb[:])

            # ---- v projection (normal layout, x^T stationary) ----
            vap = top.enter_context(tc.tile_pool(name="vap", bufs=1))
            v_all = vap.tile([HD, NCHUNK, HPC * HD], BF16)
            with ExitStack() as ph:
                vps = ph.enter_context(tc.tile_pool(name="vps", bufs=4, space="PSUM"))
                for ns in reversed(range(NSLICE)):
                    tsl = slice(ns * PROJ, (ns + 1) * PROJ)
                    xt = xtp.tile([HD, NCHUNK, PROJ], BF16, tag="xt")
                    nc.sync.dma_start(xt[:], xt_ap[:, :, tsl])
                    for tc2 in range(PROJ // HD):
                        ps = vps.tile([HD, HPC * HD], F32, tag="vps")
                        for k in range(NCHUNK):
                            nc.tensor.matmul(
                                ps[:],
                                xt[:, k, tc2 * HD:(tc2 + 1) * HD],
                                wv_sb[:, k, :],
                                start=(k == 0),
                                stop=(k == NCHUNK - 1),
                            )
                        nc.scalar.copy(v_all[:, ns * (PROJ // HD) + tc2, :], ps[:])

        # ---- attention + interleaved output projection ----
        ctxp = top.enter_context(tc.tile_pool(name="ctxp", bufs=1))
        ctx_sb = ctxp.tile([HD, HPC, T], BF16)
        wop = top.enter_context(tc.tile_pool(name="wop", bufs=1))
        wo_sb = wop.tile([HD, HPC, D], BF16)
        with ExitStack() as ph:
            ptp = ph.enter_context(tc.tile_pool(name="ptp", side="right", bufs=2))
            mkpre = ph.enter_context(tc.tile_pool(name="mkpre", side="right", bufs=1))
            lrp = ph.enter_context(tc.tile_pool(name="lrp", side="right", bufs=2))
            evp = ph.enter_context(tc.tile_pool(name="evp", side="right", bufs=4))
            sps = ph.enter_context(tc.tile_pool(name="sps", bufs=2, space="PSUM"))
            olps = ph.enter_context(tc.tile_pool(name="olps", bufs=2, space="PSUM"))
            wops = ph.enter_context(tc.tile_pool(name="wops", bufs=2, space="PSUM"))

            mk_sb = mkpre.tile([HD, nmask, PAIR], BF16)
            nc.gpsimd.dma_start(mk_sb[:], mk_d[:])
            nc.gpsimd.dma_start(wo_sb[:], wo_ap[:])  # prefetch wo

            def finalize(st):
                # DVE fast-recip of the (already partition-broadcast) row
                # sums, then one DVE multiply into ctx
                lr = lrp.tile([HD, PAIR], F32, tag="lr")
                nc.vector.reciprocal_approx_fast(lr[:], st["l"])
                nc.vector.tensor_mul(
                    ctx_sb[:, st["h"], st["qsl"]], st["o"], lr[:]
                )

            def emit_outproj(j):
                # output projection for query group j (all heads final)
                for tck in range(PAIR // HD):
                    tsl = slice(j * PAIR + tck * HD, j * PAIR + (tck + 1) * HD)
                    for es in range(D // 512):
                        esl = slice(es * 512, (es + 1) * 512)
                        ps = wops.tile([HD, 512], F32, tag="wo")
                        for h in range(HPC):
                            nc.tensor.matmul(
                                ps[:],
                                ctx_sb[:, h, tsl],
                                wo_sb[:, h, esl],
                                start=(h == 0),
                                stop=(h == HPC - 1),
                            )
                        ev = evp.tile([HD, 512], F32, tag="ev")
                        nc.vector.tensor_copy(ev[:], ps[:])
                        nc.sync.dma_start(out_d[tsl, esl], ev[:])

            def emit_ol(dq):
                # deferred p@v matmuls for an exp'd quad.  o and l share one
                # PSUM bank, so l's accumulation group must not open while
                # o's is still open: all l matmuls are emitted as one block
                # right after o's group closes.
                pi, quad, st = dq
                h = st["h"]
                for t, (c, mi) in enumerate(quad):
                    nc.tensor.matmul(
                        st["o"],
                        v_all[:, c, h * HD:(h + 1) * HD],
                        st["pt"][:, pi + t, :],
                        start=(st["oi"] == 0),
                        stop=(st["oi"] == st["n"] - 1),
                        skip_group_check=True,
                    )
                    st["oi"] += 1
                if st["oi"] < st["n"]:
                    return False
                for li in range(st["n"]):
                    nc.tensor.matmul(
                        st["l"],
                        ones_sb[:],
                        st["pt"][:, li, :],
                        start=(li == 0),
                        stop=(li == st["n"] - 1),
                        skip_group_check=True,
                    )
                return True

            pending_ol = None
            pending_fin = None
            for j in reversed(range(NPAIR)):
                qsl = slice(j * PAIR, (j + 1) * PAIR)
                chunks = list(reversed(statuses[j]))
                n = len(chunks)
                quads = [chunks[ii:ii + 4] for ii in range(0, n, 4)]
                for h in range(HPC):
                    ol_ps = olps.tile([HD, 2 * PAIR], F32, tag="ol")
                    pt = ptp.tile([HD, NCHUNK, PAIR], BF16, tag="pt")
                    st = {"o": ol_ps[:, 0:PAIR], "l": ol_ps[:, PAIR:2 * PAIR],
                          "pt": pt, "h": h, "qsl": qsl, "j": j, "n": n,
                          "oi": 0, "li": 0}
                    for qi, quad in enumerate(quads):
                        w = len(quad)
                        s_ps = sps.tile([HD, 4, PAIR], F32, tag="s")
                        for t, (c, mi) in enumerate(quad):
                            nc.tensor.matmul(
                                s_ps[:, t, :],
                                qk_sb[:, HPC + h, c * HD:(c + 1) * HD],
                                qk_sb[:, h, qsl],
                                start=True,
                                stop=True,
                            )
                        nc.scalar.activation(
                            pt[:, qi * 4:qi * 4 + w, :], s_ps[:, 0:w, :], EXP
                        )
                        # multiplicative exp-mask applied to pt
                        # (exp(s+m) == exp(s)*exp(m)), off the exp chain
                        t = 0
                        while t < w:
                            c, mi = quad[t]
                            if mi < 0:
                                t += 1
                                continue
                            r = t + 1
                            while (r < w and quad[r][1] >= 0
                                   and quad[r][1] == quad[r - 1][1] + 1):
                                r += 1
                            sl = slice(qi * 4 + t, qi * 4 + r)
                            nc.vector.tensor_mul(
                                pt[:, sl, :], pt[:, sl, :],
                                mk_sb[:, mi:mi + (r - t), :],
                            )
                            t = r
                        if pending_ol is not None:
                            if emit_ol(pending_ol):
                                pending_fin = pending_ol[2]
                            pending_ol = None
                        if pending_fin is not None and pending_fin is not st:
                            fs = pending_fin
                            finalize(fs)
                            pending_fin = None
                            if fs["h"] == HPC - 1:
                                emit_outproj(fs["j"])
                        pending_ol = (qi * 4, quad, st)
            if pending_ol is not None:
                if emit_ol(pending_ol):
                    pending_fin = pending_ol[2]
            if pending_fin is not None:
                finalize(pending_fin)
                if pending_fin["h"] == HPC - 1:
                    emit_outproj(pending_fin["j"])
    nc.compile()
    return nc


_PERM = np.concatenate(
    [np.concatenate([np.arange(0, HD, 2), np.arange(1, HD, 2)]) + h * HD
     for h in range(HPC)]
)


def prepare(x, freqs, mask, wq, wk, wv, wo):
    """Host-side sharding/prep. Returns (nc, in_maps)."""
    x = np.asarray(x, np.float32)
    freqs = np.asarray(freqs, np.float32)
    mask = np.asarray(mask, np.float32)
    wq, wk, wv, wo = (np.asarray(w, np.float32) for w in (wq, wk, wv, wo))

    statuses, maskt = _mask_structure(mask)
    nc = _build_program(statuses, maskt.shape[1])

    scale = np.float32(1.0 / np.sqrt(HD))
    cos = np.ascontiguousarray(freqs[:, :, 0].T)  # (64, T)
    sin = np.ascontiguousarray(freqs[:, :, 1].T)
    cs = np.empty((HD, 2, T), np.float32)
    cs[0:64, 0, :] = cos
    cs[64:128, 0, :] = cos
    cs[0:64, 1, :] = -sin
    cs[64:128, 1, :] = sin

    ones_sq = np.ones((HD, HD), NPBF16)
    maskt_bf = maskt.astype(NPBF16)
    xt = [np.ascontiguousarray(x[b].T).astype(NPBF16) for b in range(B)]

    in_maps = []
    for core in range(8):
        b, g = core // 4, core % 4
        cols = slice(g * HPC * HD, (g + 1) * HPC * HD)
        in_maps.append({
            "xt": xt[b],
            "wqt": np.ascontiguousarray((wq.T[:, cols] * scale)[:, _PERM]).astype(NPBF16),
            "wkt": np.ascontiguousarray(wk.T[:, cols][:, _PERM]).astype(NPBF16),
            "wvt": np.ascontiguousarray(wv.T[:, cols]).astype(NPBF16),
            "wot": np.ascontiguousarray(wo.T[cols, :]).astype(NPBF16),
            "cs": cs,
            "maskt": maskt_bf,
            "ones_sq": ones_sq,
        })
    return nc, in_maps


def run(x, freqs, mask, wq, wk, wv, wo, **spmd_kwargs):
    nc, in_maps = prepare(x, freqs, mask, wq, wk, wv, wo)
    res = run_bass_kernel_spmd(nc, in_maps, list(range(8)), **spmd_kwargs)
    parts = [res.results[c]["out"] for c in range(8)]
    out = np.stack([
        parts[b * 4] + parts[b * 4 + 1] + parts[b * 4 + 2] + parts[b * 4 + 3]
        for b in range(B)
    ]).astype(np.float32)
    return out, res


def kernel(x, freqs, mask, wq, wk, wv, wo):
    out, _ = run(x, freqs, mask, wq, wk, wv, wo)
    return out


# revision 8
# speedup vs baseline: 1.0062x; 1.0062x over previous
"""Trainium2 Bass kernel for causal self-attention with RoPE.

Shapes: x (2, 2048, 2048), 16 heads x 128 head_dim.
Sharding: 8 cores = 2 batch x 4 head-groups (4 heads per core).
Each core computes q/k/v projections for its heads, RoPE, causal-masked
softmax attention, and a partial output projection (its head columns of
wo); the host sums the 4 partials per batch element.

Layout strategy (per core):
  - all matmul operands in bf16 (halves DMA + SBUF; matmul rate is the
    same 1 row/cycle as fp32r, accumulation stays fp32 in PSUM).
  - q,k built in transposed layout (head_dim on partitions, t free) so
    RoPE and the score matmuls need no on-device transposes.  The host
    permutes wq/wk columns so RoPE's even/odd pairs become the two
    partition halves, and pre-scales wq by 1/sqrt(head_dim).
  - projections run on 512-wide query slices (half the matmul count of
    256-wide, so less exposed PE pipeline latency).
  - scores computed as s^T (keys x q) per 256-query group; softmax skips
    the max-subtraction (scores are O(1) by construction); row sums via
    a 128-wide ones matmul that lands already partition-broadcast, so
    normalization needs no gpsimd hop; o and l share one PSUM bank.
  - v computed directly in (t x e) layout by using x^T as the stationary
    operand, so the p@v matmul needs no transposes anywhere.
  - fully-masked key blocks are skipped (host inspects the mask);
    partially-masked blocks get a multiplicative exp-mask on the DVE.
  - the output projection for a query group is interleaved right after
    its last head finalizes, filling attention-phase PE bubbles instead
    of serializing at the end.
  - DMA is spread over three queues: weights wq on the scalar queue,
    wk/wv on the gpsimd queue, activations/tables/outputs on sync, so
    the first matmuls can start ~6us in.
"""

import sys
from contextlib import ExitStack

if "/opt/trn_rl_repo" not in sys.path:
    sys.path.insert(0, "/opt/trn_rl_repo")

import numpy as np
import ml_dtypes

import concourse.bacc as bacc
import concourse.mybir as mybir
import concourse.tile as tile
from concourse.bass_utils import run_bass_kernel_spmd

B, T, D, NH, HD = 2, 2048, 2048, 16, 128
HPC = 4              # heads per core
PAIR = 256           # queries per attention group
NPAIR = T // PAIR    # 8
NCHUNK = T // HD     # 16 key chunks of 128
PROJ = 512           # queries per projection slice
NSLICE = T // PROJ   # 4
BF16 = mybir.dt.bfloat16
F32 = mybir.dt.float32
NPBF16 = np.dtype(ml_dtypes.bfloat16)


def _mask_structure(mask):
    """Classify each (query-group, key-chunk) block of the additive mask.

    Returns (statuses, maskt): statuses[j] is a list of
    (chunk, mask_tile_index_or_minus1) for blocks that must be computed;
    maskt is the packed (128, nmask, 256) array of deduplicated
    transposed mask tiles for partially-masked blocks.
    """
    statuses = []
    tiles = {}
    tile_list = []
    for j in range(NPAIR):
        q = slice(j * PAIR, (j + 1) * PAIR)
        lst = []
        for c in range(NCHUNK):
            k = slice(c * HD, (c + 1) * HD)
            sub = mask[q, k]
            if np.all(sub <= -1e8):
                continue
            if np.all(sub == 0.0):
                lst.append((c, -1))
            else:
                key = sub.tobytes()
                mi = tiles.get(key)
                if mi is None:
                    mi = len(tile_list)
                    tiles[key] = mi
                    tile_list.append(np.ascontiguousarray(sub.T))
                lst.append((c, mi))
        assert lst, f"query group {j} has every key block masked"
        statuses.append(lst)
    nmask = max(1, len(tile_list))
    maskt = np.zeros((HD, nmask, PAIR), np.float32)
    for i, t in enumerate(tile_list):
        assert np.all(t <= 64.0), "additive mask too large for exp-mask trick"
        maskt[:, i, :] = np.exp(t)
    return statuses, maskt


def _build_program(statuses, nmask):
    nc = bacc.Bacc(None, target_bir_lowering=False)

    xt_d = nc.dram_tensor("xt", [D, T], BF16, kind="ExternalInput")
    wq_d = nc.dram_tensor("wqt", [D, HPC * HD], BF16, kind="ExternalInput")
    wk_d = nc.dram_tensor("wkt", [D, HPC * HD], BF16, kind="ExternalInput")
    wv_d = nc.dram_tensor("wvt", [D, HPC * HD], BF16, kind="ExternalInput")
    wo_d = nc.dram_tensor("wot", [HPC * HD, D], BF16, kind="ExternalInput")
    cs_d = nc.dram_tensor("cs", [HD, 2, T], F32, kind="ExternalInput")
    mk_d = nc.dram_tensor("maskt", [HD, nmask, PAIR], BF16, kind="ExternalInput")
    ones_d = nc.dram_tensor("ones_sq", [HD, HD], BF16, kind="ExternalInput")
    out_d = nc.dram_tensor("out", [T, D], F32, kind="ExternalOutput")

    xt_ap = xt_d.ap().rearrange("(k p) t -> p k t", p=HD)
    wq_ap = wq_d.ap().rearrange("(k p) e -> p k e", p=HD)
    wk_ap = wk_d.ap().rearrange("(k p) e -> p k e", p=HD)
    wv_ap = wv_d.ap().rearrange("(k p) e -> p k e", p=HD)
    wo_ap = wo_d.ap().rearrange("(h p) e -> p h e", p=HD)
    EXP = mybir.ActivationFunctionType.Exp

    with tile.TileContext(nc) as tc, ExitStack() as top:
        constp = top.enter_context(tc.tile_pool(name="const", bufs=1))
        ones_sb = constp.tile([HD, HD], BF16)
        nc.gpsimd.dma_start(ones_sb[:], ones_d[:])

        qkp = top.enter_context(tc.tile_pool(name="qkp", bufs=1))
        # q heads at [:, h, :], k heads at [:, 4+h, :]
        qk_sb = qkp.tile([HD, 2 * HPC, T], BF16)

        # wv + xt pools span the q/k pass (prefetch) and the v pass
        with ExitStack() as vph:
            wvp = vph.enter_context(tc.tile_pool(name="wvp", side="right", bufs=1))
            wv_sb = wvp.tile([HD, NCHUNK, HPC * HD], BF16)
            xtp = vph.enter_context(tc.tile_pool(name="xtp", side="right", bufs=2))

            # ---- combined q/k projection pass (+ fused RoPE) ----
            with ExitStack() as ph:
                wp = ph.enter_context(tc.tile_pool(name="wp", side="right", bufs=1))
                csp = ph.enter_context(tc.tile_pool(name="csp", side="right", bufs=2))
                ropep = ph.enter_context(tc.tile_pool(name="ropep", side="right", bufs=2))
                pps = ph.enter_context(tc.tile_pool(name="pps", bufs=6, space="PSUM"))
                wqk_sb = wp.tile([HD, 2, NCHUNK, HPC * HD], BF16)
                # DMA issue overhead (~0.6us per dma_start on the issuing
                # engine) dominates startup if transfers are split fine, so
                # everything moves as one DMA per tensor, ordered by first
                # use: wq alone on the scalar queue (lands ~9us); the sync
                # queue carries xt slice 0, then wk, then the remaining
                # slices; wv/wo/mask/ones ride the gpsimd SWDGE queue since
                # they aren't needed until the v pass / attention.
                nc.scalar.dma_start(wqk_sb[:, 0, :, :], wq_ap[:])
                nc.gpsimd.dma_start(wv_sb[:], wv_ap[:])
                for ns in range(NSLICE):
                    tsl = slice(ns * PROJ, (ns + 1) * PROJ)
                    xt = xtp.tile([HD, NCHUNK, PROJ], BF16, tag="xt")
                    nc.sync.dma_start(xt[:], xt_ap[:, :, tsl])
                    if ns == 0:
                        nc.sync.dma_start(wqk_sb[:, 1, :, :], wk_ap[:])
                    cs_sl = csp.tile([HD, 2, PROJ], F32, tag="cs")
                    nc.sync.dma_start(cs_sl[:], cs_d[:, :, tsl])
                    for wsel in range(2):
                        for h in range(HPC):
                            ps = pps.tile([HD, PROJ], F32, tag="ps")
                            hs = slice(h * HD, (h + 1) * HD)
                            for k in range(NCHUNK):
                                nc.tensor.matmul(
                                    ps[:],
                                    wqk_sb[:, wsel, k, hs],
                                    xt[:, k, :],
                                    start=(k == 0),
                                    stop=(k == NCHUNK - 1),
                                )
                            # RoPE: dst = raw*C + swap(raw)*S.  The swap is
                            # materialized by two ScalarE half-copies; VectorE
                            # does two multiplies and one add per tile.
                            dst = qk_sb[:, wsel * HPC + h, tsl]
                            sw = ropep.tile([HD, PROJ], F32, tag="sw")
                            nc.scalar.copy(sw[0:64, :], ps[64:128, :])
                            nc.scalar.copy(sw[64:128, :], ps[0:64, :])
                            tb = ropep.tile([HD, PROJ], F32, tag="tb")
                            nc.vector.tensor_mul(dst, ps[:], cs_sl[:, 0, :])
                            nc.vector.tensor_mul(tb[:], sw[:], cs_sl[:, 1, :])
                            nc.vector.tensor_add(dst, dst, tb[:])

            # ---- v projection (normal layout, x^T stationary) ----
            vap = top.enter_context(tc.tile_pool(name="vap", bufs=1))
            v_all = vap.tile([HD, NCHUNK, HPC * HD], BF16)
            with ExitStack() as ph:
                vps = ph.enter_context(tc.tile_pool(name="vps", bufs=6, space="PSUM"))
                for ns in reversed(range(NSLICE)):
                    tsl = slice(ns * PROJ, (ns + 1) * PROJ)
                    xt = xtp.tile([HD, NCHUNK, PROJ], BF16, tag="xt")
                    nc.sync.dma_start(xt[:], xt_ap[:, :, tsl])
                    for tc2 in range(PROJ // HD):
                        ps = vps.tile([HD, HPC * HD], F32, tag="vps")
                        for k in range(NCHUNK):
                            nc.tensor.matmul(
                                ps[:],
                                xt[:, k, tc2 * HD:(tc2 + 1) * HD],
                                wv_sb[:, k, :],
                                start=(k == 0),
                                stop=(k == NCHUNK - 1),
                            )
                        nc.scalar.copy(v_all[:, ns * (PROJ // HD) + tc2, :], ps[:])

        # ---- attention + interleaved output projection ----
        ctxp = top.enter_context(tc.tile_pool(name="ctxp", bufs=1))
        ctx_sb = ctxp.tile([HD, HPC, T], BF16)
        wop = top.enter_context(tc.tile_pool(name="wop", bufs=1))
        wo_sb = wop.tile([HD, HPC, D], BF16)
        with ExitStack() as ph:
            ptp = ph.enter_context(tc.tile_pool(name="ptp", side="right", bufs=2))
            mkpre = ph.enter_context(tc.tile_pool(name="mkpre", side="right", bufs=1))
            lrp = ph.enter_context(tc.tile_pool(name="lrp", side="right", bufs=2))
            evp = ph.enter_context(tc.tile_pool(name="evp", side="right", bufs=4))
            sps = ph.enter_context(tc.tile_pool(name="sps", bufs=2, space="PSUM"))
            olps = ph.enter_context(tc.tile_pool(name="olps", bufs=2, space="PSUM"))
            wops = ph.enter_context(tc.tile_pool(name="wops", bufs=2, space="PSUM"))

            mk_sb = mkpre.tile([HD, nmask, PAIR], BF16)
            nc.gpsimd.dma_start(mk_sb[:], mk_d[:])
            nc.gpsimd.dma_start(wo_sb[:], wo_ap[:])  # prefetch wo

            def finalize(st):
                # DVE fast-recip of the (already partition-broadcast) row
                # sums, then one DVE multiply into ctx
                lr = lrp.tile([HD, PAIR], F32, tag="lr")
                nc.vector.reciprocal_approx_fast(lr[:], st["l"])
                nc.vector.tensor_mul(
                    ctx_sb[:, st["h"], st["qsl"]], st["o"], lr[:]
                )

            def emit_outproj(j):
                # output projection for query group j (all heads final)
                for tck in range(PAIR // HD):
                    tsl = slice(j * PAIR + tck * HD, j * PAIR + (tck + 1) * HD)
                    for es in range(D // 512):
                        esl = slice(es * 512, (es + 1) * 512)
                        ps = wops.tile([HD, 512], F32, tag="wo")
                        for h in range(HPC):
                            nc.tensor.matmul(
                                ps[:],
                                ctx_sb[:, h, tsl],
                                wo_sb[:, h, esl],
                                start=(h == 0),
                                stop=(h == HPC - 1),
                            )
                        ev = evp.tile([HD, 512], F32, tag="ev")
                        nc.vector.tensor_copy(ev[:], ps[:])
                        nc.sync.dma_start(out_d[tsl, esl], ev[:])

            def emit_ol(dq):
                # deferred p@v matmuls for an exp'd quad.  o and l share one
                # PSUM bank, so l's accumulation group must not open while
                # o's is still open: all l matmuls are emitted as one block
                # right after o's group closes.
                pi, quad, st = dq
                h = st["h"]
                for t, (c, mi) in enumerate(quad):
                    nc.tensor.matmul(
                        st["o"],
                        v_all[:, c, h * HD:(h + 1) * HD],
                        st["pt"][:, pi + t, :],
                        start=(st["oi"] == 0),
                        stop=(st["oi"] == st["n"] - 1),
                        skip_group_check=True,
                    )
                    st["oi"] += 1
                if st["oi"] < st["n"]:
                    return False
                for li in range(st["n"]):
                    nc.tensor.matmul(
                        st["l"],
                        ones_sb[:],
                        st["pt"][:, li, :],
                        start=(li == 0),
                        stop=(li == st["n"] - 1),
                        skip_group_check=True,
                    )
                return True

            pending_ol = None
            pending_fin = None
            for j in reversed(range(NPAIR)):
                qsl = slice(j * PAIR, (j + 1) * PAIR)
                chunks = list(reversed(statuses[j]))
                n = len(chunks)
                quads = [chunks[ii:ii + 4] for ii in range(0, n, 4)]
                for h in range(HPC):
                    ol_ps = olps.tile([HD, 2 * PAIR], F32, tag="ol")
                    pt = ptp.tile([HD, NCHUNK, PAIR], BF16, tag="pt")
                    st = {"o": ol_ps[:, 0:PAIR], "l": ol_ps[:, PAIR:2 * PAIR],
                          "pt": pt, "h": h, "qsl": qsl, "j": j, "n": n,
                          "oi": 0, "li": 0}
                    for qi, quad in enumerate(quads):
                        w = len(quad)
                        s_ps = sps.tile([HD, 4, PAIR], F32, tag="s")
                        for t, (c, mi) in enumerate(quad):
                            nc.tensor.matmul(
                                s_ps[:, t, :],
                                qk_sb[:, HPC + h, c * HD:(c + 1) * HD],
                                qk_sb[:, h, qsl],
                                start=True,
                                stop=True,
                            )
                        nc.scalar.activation(
                            pt[:, qi * 4:qi * 4 + w, :], s_ps[:, 0:w, :], EXP
                        )
                        # multiplicative exp-mask applied to pt
                        # (exp(s+m) == exp(s)*exp(m)), off the exp chain
                        t = 0
                        while t < w:
                            c, mi = quad[t]
                            if mi < 0:
                                t += 1
                                continue
                            r = t + 1
                            while (r < w and quad[r][1] >= 0
                                   and quad[r][1] == quad[r - 1][1] + 1):
                                r += 1
                            sl = slice(qi * 4 + t, qi * 4 + r)
                            nc.vector.tensor_mul(
                                pt[:, sl, :], pt[:, sl, :],
                                mk_sb[:, mi:mi + (r - t), :],
                            )
                            t = r
                        if pending_ol is not None:
                            if emit_ol(pending_ol):
                                pending_fin = pending_ol[2]
                            pending_ol = None
                        if pending_fin is not None and pending_fin is not st:
                            fs = pending_fin
                            finalize(fs)
                            pending_fin = None
                            if fs["h"] == HPC - 1:
                                emit_outproj(fs["j"])
                        pending_ol = (qi * 4, quad, st)
            if pending_ol is not None:
                if emit_ol(pending_ol):
                    pending_fin = pending_ol[2]
            if pending_fin is not None:
                finalize(pending_fin)
                if pending_fin["h"] == HPC - 1:
                    emit_outproj(pending_fin["j"])
    nc.compile()
    return nc


_PERM = np.concatenate(
    [np.concatenate([np.arange(0, HD, 2), np.arange(1, HD, 2)]) + h * HD
     for h in range(HPC)]
)


def prepare(x, freqs, mask, wq, wk, wv, wo):
    """Host-side sharding/prep. Returns (nc, in_maps)."""
    x = np.asarray(x, np.float32)
    freqs = np.asarray(freqs, np.float32)
    mask = np.asarray(mask, np.float32)
    wq, wk, wv, wo = (np.asarray(w, np.float32) for w in (wq, wk, wv, wo))

    statuses, maskt = _mask_structure(mask)
    nc = _build_program(statuses, maskt.shape[1])

    scale = np.float32(1.0 / np.sqrt(HD))
    cos = np.ascontiguousarray(freqs[:, :, 0].T)  # (64, T)
    sin = np.ascontiguousarray(freqs[:, :, 1].T)
    cs = np.empty((HD, 2, T), np.float32)
    cs[0:64, 0, :] = cos
    cs[64:128, 0, :] = cos
    cs[0:64, 1, :] = -sin
    cs[64:128, 1, :] = sin

    ones_sq = np.ones((HD, HD), NPBF16)
    maskt_bf = maskt.astype(NPBF16)
    xt = [np.ascontiguousarray(x[b].T).astype(NPBF16) for b in range(B)]

    in_maps = []
    for core in range(8):
        b, g = core // 4, core % 4
        cols = slice(g * HPC * HD, (g + 1) * HPC * HD)
        in_maps.append({
            "xt": xt[b],
            "wqt": np.ascontiguousarray((wq.T[:, cols] * scale)[:, _PERM]).astype(NPBF16),
            "wkt": np.ascontiguousarray(wk.T[:, cols][:, _PERM]).astype(NPBF16),
            "wvt": np.ascontiguousarray(wv.T[:, cols]).astype(NPBF16),
            "wot": np.ascontiguousarray(wo.T[cols, :]).astype(NPBF16),
            "cs": cs,
            "maskt": maskt_bf,
            "ones_sq": ones_sq,
        })
    return nc, in_maps


def run(x, freqs, mask, wq, wk, wv, wo, **spmd_kwargs):
    nc, in_maps = prepare(x, freqs, mask, wq, wk, wv, wo)
    res = run_bass_kernel_spmd(nc, in_maps, list(range(8)), **spmd_kwargs)
    parts = [res.results[c]["out"] for c in range(8)]
    out = np.stack([
        parts[b * 4] + parts[b * 4 + 1] + parts[b * 4 + 2] + parts[b * 4 + 3]
        for b in range(B)
    ]).astype(np.float32)
    return out, res


def kernel(x, freqs, mask, wq, wk, wv, wo):
    out, _ = run(x, freqs, mask, wq, wk, wv, wo)
    return out


# revision 12
# speedup vs baseline: 1.0359x; 1.0296x over previous
"""Trainium2 Bass kernel for causal self-attention with RoPE.

Shapes: x (2, 2048, 2048), 16 heads x 128 head_dim.
Sharding: 8 cores = 2 batch x 4 head-groups (4 heads per core).
Each core computes q/k/v projections for its heads, RoPE, causal-masked
softmax attention, and a partial output projection (its head columns of
wo); the host sums the 4 partials per batch element.

Layout strategy (per core):
  - all matmul operands in bf16 (halves DMA + SBUF; matmul rate is the
    same 1 row/cycle as fp32r, accumulation stays fp32 in PSUM).
  - q,k built in transposed layout (head_dim on partitions, t free) so
    RoPE and the score matmuls need no on-device transposes.  The host
    permutes wq/wk columns so RoPE's even/odd pairs become the two
    partition halves, and pre-scales wq by 1/sqrt(head_dim).
  - projections run on 512-wide query slices (half the matmul count of
    256-wide, so less exposed PE pipeline latency).
  - scores computed as s^T (keys x q) per 256-query group; softmax skips
    the max-subtraction (scores are O(1) by construction); row sums via
    a 128-wide ones matmul that lands already partition-broadcast, so
    normalization needs no gpsimd hop; o and l share one PSUM bank.
  - v computed directly in (t x e) layout by using x^T as the stationary
    operand, so the p@v matmul needs no transposes anywhere.
  - fully-masked key blocks are skipped (host inspects the mask);
    partially-masked blocks get a multiplicative exp-mask on the DVE.
  - the output projection for a query group is interleaved right after
    its last head finalizes, filling attention-phase PE bubbles instead
    of serializing at the end.
  - DMA is spread over three queues: weights wq on the scalar queue,
    wk/wv on the gpsimd queue, activations/tables/outputs on sync, so
    the first matmuls can start ~6us in.
"""

import sys
from contextlib import ExitStack

if "/opt/trn_rl_repo" not in sys.path:
    sys.path.insert(0, "/opt/trn_rl_repo")

import numpy as np
import ml_dtypes

import concourse.bacc as bacc
import concourse.mybir as mybir
import concourse.tile as tile
from concourse.bass_utils import run_bass_kernel_spmd

B, T, D, NH, HD = 2, 2048, 2048, 16, 128
HPC = 4              # heads per core
PAIR = 256           # queries per attention group
NPAIR = T // PAIR    # 8
NCHUNK = T // HD     # 16 key chunks of 128
PROJ = 512           # queries per projection slice
NSLICE = T // PROJ   # 4
BF16 = mybir.dt.bfloat16
F32 = mybir.dt.float32
NPBF16 = np.dtype(ml_dtypes.bfloat16)


def _mask_structure(mask):
    """Classify each (query-group, key-chunk) block of the additive mask.

    Returns (statuses, maskt): statuses[j] is a list of
    (chunk, mask_tile_index_or_minus1) for blocks that must be computed;
    maskt is the packed (128, nmask, 256) array of deduplicated
    transposed mask tiles for partially-masked blocks.
    """
    statuses = []
    tiles = {}
    tile_list = []
    for j in range(NPAIR):
        q = slice(j * PAIR, (j + 1) * PAIR)
        lst = []
        for c in range(NCHUNK):
            k = slice(c * HD, (c + 1) * HD)
            sub = mask[q, k]
            if np.all(sub <= -1e8):
                continue
            if np.all(sub == 0.0):
                lst.append((c, -1))
            else:
                key = sub.tobytes()
                mi = tiles.get(key)
                if mi is None:
                    mi = len(tile_list)
                    tiles[key] = mi
                    tile_list.append(np.ascontiguousarray(sub.T))
                lst.append((c, mi))
        assert lst, f"query group {j} has every key block masked"
        statuses.append(lst)
    nmask = max(1, len(tile_list))
    maskt = np.zeros((HD, nmask, PAIR), np.float32)
    for i, t in enumerate(tile_list):
        assert np.all(t <= 64.0), "additive mask too large for exp-mask trick"
        maskt[:, i, :] = np.exp(t)
    return statuses, maskt


def _build_program(statuses, nmask):
    nc = bacc.Bacc(None, target_bir_lowering=False)

    # All inputs are laid out on the host so that each SBUF partition's
    # content is one contiguous DRAM run (16KB descriptors).  Fine-grained
    # (1KB) descriptors cap a DMA queue at ~80GB/s because of per-descriptor
    # processing overhead; 16KB descriptors run at the full bus rate.
    xt_d = nc.dram_tensor("xts", [NSLICE, HD, NCHUNK, PROJ], BF16,
                          kind="ExternalInput")
    wq_d = nc.dram_tensor("wqt", [HD, NCHUNK, HPC * HD], BF16, kind="ExternalInput")
    wk_d = nc.dram_tensor("wkt", [HD, NCHUNK, HPC * HD], BF16, kind="ExternalInput")
    wv_d = nc.dram_tensor("wvt", [HD, NCHUNK, HPC * HD], BF16, kind="ExternalInput")
    wo_d = nc.dram_tensor("wot", [HD, HPC, D], BF16, kind="ExternalInput")
    cs_d = nc.dram_tensor("cs", [HD, 2, T], F32, kind="ExternalInput")
    mk_d = nc.dram_tensor("maskt", [HD, nmask, PAIR], BF16, kind="ExternalInput")
    ones_d = nc.dram_tensor("ones_sq", [HD, HD], BF16, kind="ExternalInput")
    out_d = nc.dram_tensor("out", [T, D], F32, kind="ExternalOutput")

    xt_ap = xt_d.ap()
    wq_ap = wq_d.ap()
    wk_ap = wk_d.ap()
    wv_ap = wv_d.ap()
    wo_ap = wo_d.ap()
    EXP = mybir.ActivationFunctionType.Exp

    with tile.TileContext(nc) as tc, ExitStack() as top:
        constp = top.enter_context(tc.tile_pool(name="const", bufs=1))
        ones_sb = constp.tile([HD, HD], BF16)
        nc.gpsimd.dma_start(ones_sb[:], ones_d[:])

        qkp = top.enter_context(tc.tile_pool(name="qkp", bufs=1))
        # q heads at [:, h, :], k heads at [:, 4+h, :]
        qk_sb = qkp.tile([HD, 2 * HPC, T], BF16)

        # wv + xt pools span the q/k pass (prefetch) and the v pass
        with ExitStack() as vph:
            wvp = vph.enter_context(tc.tile_pool(name="wvp", side="right", bufs=1))
            wv_sb = wvp.tile([HD, NCHUNK, HPC * HD], BF16)
            xtp = vph.enter_context(tc.tile_pool(name="xtp", side="right", bufs=2))

            # ---- combined q/k projection pass (+ fused RoPE) ----
            with ExitStack() as ph:
                wp = ph.enter_context(tc.tile_pool(name="wp", side="right", bufs=1))
                csp = ph.enter_context(tc.tile_pool(name="csp", side="right", bufs=2))
                ropep = ph.enter_context(tc.tile_pool(name="ropep", side="right", bufs=2))
                pps = ph.enter_context(tc.tile_pool(name="pps", bufs=6, space="PSUM"))
                wqk_sb = wp.tile([HD, 2, NCHUNK, HPC * HD], BF16)
                # DMA issue overhead (~0.6us per dma_start on the issuing
                # engine) dominates startup if transfers are split fine, so
                # everything moves as one DMA per tensor, ordered by first
                # use: wq alone on the scalar queue (lands ~9us); the sync
                # queue carries xt slice 0, then wk, then the remaining
                # slices; wv/wo/mask/ones ride the gpsimd SWDGE queue since
                # they aren't needed until the v pass / attention.
                nc.scalar.dma_start(wqk_sb[:, 0, :, :], wq_ap[:])
                nc.gpsimd.dma_start(wv_sb[:], wv_ap[:])
                for ns in range(NSLICE):
                    tsl = slice(ns * PROJ, (ns + 1) * PROJ)
                    xt = xtp.tile([HD, NCHUNK, PROJ], BF16, tag="xt")
                    nc.sync.dma_start(xt[:], xt_ap[ns])
                    if ns == 0:
                        nc.sync.dma_start(wqk_sb[:, 1, :, :], wk_ap[:])
                    cs_sl = csp.tile([HD, 2, PROJ], F32, tag="cs")
                    nc.sync.dma_start(cs_sl[:], cs_d[:, :, tsl])
                    for wsel in range(2):
                        for h in range(HPC):
                            ps = pps.tile([HD, PROJ], F32, tag="ps")
                            hs = slice(h * HD, (h + 1) * HD)
                            for k in range(NCHUNK):
                                nc.tensor.matmul(
                                    ps[:],
                                    wqk_sb[:, wsel, k, hs],
                                    xt[:, k, :],
                                    start=(k == 0),
                                    stop=(k == NCHUNK - 1),
                                )
                            # RoPE: dst = raw*C + swap(raw)*S.  The swap is
                            # materialized by two ScalarE half-copies; VectorE
                            # does two multiplies and one add per tile.
                            dst = qk_sb[:, wsel * HPC + h, tsl]
                            sw = ropep.tile([HD, PROJ], F32, tag="sw")
                            nc.scalar.copy(sw[0:64, :], ps[64:128, :])
                            nc.scalar.copy(sw[64:128, :], ps[0:64, :])
                            tb = ropep.tile([HD, PROJ], F32, tag="tb")
                            nc.vector.tensor_mul(dst, ps[:], cs_sl[:, 0, :])
                            nc.vector.tensor_mul(tb[:], sw[:], cs_sl[:, 1, :])
                            nc.vector.tensor_add(dst, dst, tb[:])

            # ---- v projection (normal layout, x^T stationary) ----
            vap = top.enter_context(tc.tile_pool(name="vap", bufs=1))
            v_all = vap.tile([HD, NCHUNK, HPC * HD], BF16)
            with ExitStack() as ph:
                vps = ph.enter_context(tc.tile_pool(name="vps", bufs=6, space="PSUM"))
                for ns in reversed(range(NSLICE)):
                    xt = xtp.tile([HD, NCHUNK, PROJ], BF16, tag="xt")
                    nc.sync.dma_start(xt[:], xt_ap[ns])
                    for tc2 in range(PROJ // HD):
                        ps = vps.tile([HD, HPC * HD], F32, tag="vps")
                        for k in range(NCHUNK):
                            nc.tensor.matmul(
                                ps[:],
                                xt[:, k, tc2 * HD:(tc2 + 1) * HD],
                                wv_sb[:, k, :],
                                start=(k == 0),
                                stop=(k == NCHUNK - 1),
                            )
                        nc.scalar.copy(v_all[:, ns * (PROJ // HD) + tc2, :], ps[:])

        # ---- attention + interleaved output projection ----
        ctxp = top.enter_context(tc.tile_pool(name="ctxp", bufs=1))
        ctx_sb = ctxp.tile([HD, HPC, T], BF16)
        wop = top.enter_context(tc.tile_pool(name="wop", bufs=1))
        wo_sb = wop.tile([HD, HPC, D], BF16)
        with ExitStack() as ph:
            ptp = ph.enter_context(tc.tile_pool(name="ptp", side="right", bufs=2))
            mkpre = ph.enter_context(tc.tile_pool(name="mkpre", side="right", bufs=1))
            lrp = ph.enter_context(tc.tile_pool(name="lrp", side="right", bufs=2))
            evp = ph.enter_context(tc.tile_pool(name="evp", side="right", bufs=4))
            sps = ph.enter_context(tc.tile_pool(name="sps", bufs=2, space="PSUM"))
            olps = ph.enter_context(tc.tile_pool(name="olps", bufs=2, space="PSUM"))
            wops = ph.enter_context(tc.tile_pool(name="wops", bufs=2, space="PSUM"))

            mk_sb = mkpre.tile([HD, nmask, PAIR], BF16)
            nc.gpsimd.dma_start(mk_sb[:], mk_d[:])
            nc.gpsimd.dma_start(wo_sb[:], wo_ap[:])  # prefetch wo

            def finalize(st):
                # DVE fast-recip of the (already partition-broadcast) row
                # sums, then one DVE multiply into ctx
                lr = lrp.tile([HD, PAIR], F32, tag="lr")
                nc.vector.reciprocal_approx_fast(lr[:], st["l"])
                nc.vector.tensor_mul(
                    ctx_sb[:, st["h"], st["qsl"]], st["o"], lr[:]
                )

            def emit_outproj(j):
                # output projection for query group j (all heads final)
                for tck in range(PAIR // HD):
                    tsl = slice(j * PAIR + tck * HD, j * PAIR + (tck + 1) * HD)
                    for es in range(D // 512):
                        esl = slice(es * 512, (es + 1) * 512)
                        ps = wops.tile([HD, 512], F32, tag="wo")
                        for h in range(HPC):
                            nc.tensor.matmul(
                                ps[:],
                                ctx_sb[:, h, tsl],
                                wo_sb[:, h, esl],
                                start=(h == 0),
                                stop=(h == HPC - 1),
                            )
                        ev = evp.tile([HD, 512], F32, tag="ev")
                        nc.vector.tensor_copy(ev[:], ps[:])
                        nc.sync.dma_start(out_d[tsl, esl], ev[:])

            def emit_ol(dq):
                # deferred p@v matmuls for an exp'd quad.  o and l share one
                # PSUM bank, so l's accumulation group must not open while
                # o's is still open: all l matmuls are emitted as one block
                # right after o's group closes.
                pi, quad, st = dq
                h = st["h"]
                for t, (c, mi) in enumerate(quad):
                    nc.tensor.matmul(
                        st["o"],
                        v_all[:, c, h * HD:(h + 1) * HD],
                        st["pt"][:, pi + t, :],
                        start=(st["oi"] == 0),
                        stop=(st["oi"] == st["n"] - 1),
                        skip_group_check=True,
                    )
                    st["oi"] += 1
                if st["oi"] < st["n"]:
                    return False
                for li in range(st["n"]):
                    nc.tensor.matmul(
                        st["l"],
                        ones_sb[:],
                        st["pt"][:, li, :],
                        start=(li == 0),
                        stop=(li == st["n"] - 1),
                        skip_group_check=True,
                    )
                return True

            pending_ol = None
            pending_fin = None
            for j in reversed(range(NPAIR)):
                qsl = slice(j * PAIR, (j + 1) * PAIR)
                chunks = list(reversed(statuses[j]))
                n = len(chunks)
                quads = [chunks[ii:ii + 4] for ii in range(0, n, 4)]
                for h in range(HPC):
                    ol_ps = olps.tile([HD, 2 * PAIR], F32, tag="ol")
                    pt = ptp.tile([HD, NCHUNK, PAIR], BF16, tag="pt")
                    st = {"o": ol_ps[:, 0:PAIR], "l": ol_ps[:, PAIR:2 * PAIR],
                          "pt": pt, "h": h, "qsl": qsl, "j": j, "n": n,
                          "oi": 0, "li": 0}
                    for qi, quad in enumerate(quads):
                        w = len(quad)
                        s_ps = sps.tile([HD, 4, PAIR], F32, tag="s")
                        for t, (c, mi) in enumerate(quad):
                            nc.tensor.matmul(
                                s_ps[:, t, :],
                                qk_sb[:, HPC + h, c * HD:(c + 1) * HD],
                                qk_sb[:, h, qsl],
                                start=True,
                                stop=True,
                            )
                        nc.scalar.activation(
                            pt[:, qi * 4:qi * 4 + w, :], s_ps[:, 0:w, :], EXP
                        )
                        # multiplicative exp-mask applied to pt
                        # (exp(s+m) == exp(s)*exp(m)), off the exp chain
                        t = 0
                        while t < w:
                            c, mi = quad[t]
                            if mi < 0:
                                t += 1
                                continue
                            r = t + 1
                            while (r < w and quad[r][1] >= 0
                                   and quad[r][1] == quad[r - 1][1] + 1):
                                r += 1
                            sl = slice(qi * 4 + t, qi * 4 + r)
                            nc.vector.tensor_mul(
                                pt[:, sl, :], pt[:, sl, :],
                                mk_sb[:, mi:mi + (r - t), :],
                            )
                            t = r
                        if pending_ol is not None:
                            if emit_ol(pending_ol):
                                pending_fin = pending_ol[2]
                            pending_ol = None
                        if pending_fin is not None and pending_fin is not st:
                            fs = pending_fin
                            finalize(fs)
                            pending_fin = None
                            if fs["h"] == HPC - 1:
                                emit_outproj(fs["j"])
                        pending_ol = (qi * 4, quad, st)
            if pending_ol is not None:
                if emit_ol(pending_ol):
                    pending_fin = pending_ol[2]
            if pending_fin is not None:
                finalize(pending_fin)
                if pending_fin["h"] == HPC - 1:
                    emit_outproj(pending_fin["j"])
    nc.compile()
    return nc


_PERM = np.concatenate(
    [np.concatenate([np.arange(0, HD, 2), np.arange(1, HD, 2)]) + h * HD
     for h in range(HPC)]
)


def prepare(x, freqs, mask, wq, wk, wv, wo):
    """Host-side sharding/prep. Returns (nc, in_maps)."""
    x = np.asarray(x, np.float32)
    freqs = np.asarray(freqs, np.float32)
    mask = np.asarray(mask, np.float32)
    wq, wk, wv, wo = (np.asarray(w, np.float32) for w in (wq, wk, wv, wo))

    statuses, maskt = _mask_structure(mask)
    nc = _build_program(statuses, maskt.shape[1])

    scale = np.float32(1.0 / np.sqrt(HD))
    cos = np.ascontiguousarray(freqs[:, :, 0].T)  # (64, T)
    sin = np.ascontiguousarray(freqs[:, :, 1].T)
    cs = np.empty((HD, 2, T), np.float32)
    cs[0:64, 0, :] = cos
    cs[64:128, 0, :] = cos
    cs[0:64, 1, :] = -sin
    cs[64:128, 1, :] = sin

    ones_sq = np.ones((HD, HD), NPBF16)
    maskt_bf = maskt.astype(NPBF16)

    def pshuf_w(w):
        # [D, E] -> [HD, NCHUNK, E]: partition-major so each SBUF
        # partition's content is one contiguous DRAM run
        return np.ascontiguousarray(
            w.reshape(NCHUNK, HD, w.shape[1]).transpose(1, 0, 2)
        ).astype(NPBF16)

    def pshuf_x(xb):
        # x[b].T [D, T] -> [NSLICE, HD, NCHUNK, PROJ]
        xT = xb.T.reshape(NCHUNK, HD, NSLICE, PROJ)
        return np.ascontiguousarray(xT.transpose(2, 1, 0, 3)).astype(NPBF16)

    xts = [pshuf_x(x[b]) for b in range(B)]

    in_maps = []
    for core in range(8):
        b, g = core // 4, core % 4
        cols = slice(g * HPC * HD, (g + 1) * HPC * HD)
        wot = wo.T[cols, :].reshape(HPC, HD, D).transpose(1, 0, 2)
        in_maps.append({
            "xts": xts[b],
            "wqt": pshuf_w((wq.T[:, cols] * scale)[:, _PERM]),
            "wkt": pshuf_w(wk.T[:, cols][:, _PERM]),
            "wvt": pshuf_w(wv.T[:, cols]),
            "wot": np.ascontiguousarray(wot).astype(NPBF16),
            "cs": cs,
            "maskt": maskt_bf,
            "ones_sq": ones_sq,
        })
    return nc, in_maps


def run(x, freqs, mask, wq, wk, wv, wo, **spmd_kwargs):
    nc, in_maps = prepare(x, freqs, mask, wq, wk, wv, wo)
    res = run_bass_kernel_spmd(nc, in_maps, list(range(8)), **spmd_kwargs)
    parts = [res.results[c]["out"] for c in range(8)]
    out = np.stack([
        parts[b * 4] + parts[b * 4 + 1] + parts[b * 4 + 2] + parts[b * 4 + 3]
        for b in range(B)
    ]).astype(np.float32)
    return out, res


def kernel(x, freqs, mask, wq, wk, wv, wo):
    out, _ = run(x, freqs, mask, wq, wk, wv, wo)
    return out


# revision 15
# speedup vs baseline: 1.0451x; 1.0088x over previous
"""Trainium2 Bass kernel for causal self-attention with RoPE.

Shapes: x (2, 2048, 2048), 16 heads x 128 head_dim.
Sharding: 8 cores = 2 batch x 4 head-groups (4 heads per core).
Each core computes q/k/v projections for its heads, RoPE, causal-masked
softmax attention, and a partial output projection (its head columns of
wo); the host sums the 4 partials per batch element.

Layout strategy (per core):
  - all matmul operands in bf16 (halves DMA + SBUF; matmul rate is the
    same 1 row/cycle as fp32r, accumulation stays fp32 in PSUM).
  - q,k built in transposed layout (head_dim on partitions, t free) so
    RoPE and the score matmuls need no on-device transposes.  The host
    permutes wq/wk columns so RoPE's even/odd pairs become the two
    partition halves, and pre-scales wq by 1/sqrt(head_dim).
  - projections run on 512-wide query slices (half the matmul count of
    256-wide, so less exposed PE pipeline latency).
  - scores computed as s^T (keys x q) per 256-query group; softmax skips
    the max-subtraction (scores are O(1) by construction); row sums via
    a 128-wide ones matmul that lands already partition-broadcast, so
    normalization needs no gpsimd hop; o and l share one PSUM bank.
  - v computed directly in (t x e) layout by using x^T as the stationary
    operand, so the p@v matmul needs no transposes anywhere.
  - fully-masked key blocks are skipped (host inspects the mask);
    partially-masked blocks get a multiplicative exp-mask on the DVE.
  - the output projection for a query group is interleaved right after
    its last head finalizes, filling attention-phase PE bubbles instead
    of serializing at the end.
  - DMA is spread over three queues: weights wq on the scalar queue,
    wk/wv on the gpsimd queue, activations/tables/outputs on sync, so
    the first matmuls can start ~6us in.
"""

import sys
from contextlib import ExitStack

if "/opt/trn_rl_repo" not in sys.path:
    sys.path.insert(0, "/opt/trn_rl_repo")

import numpy as np
import ml_dtypes

import concourse.bacc as bacc
import concourse.mybir as mybir
import concourse.tile as tile
from concourse.bass_utils import run_bass_kernel_spmd

B, T, D, NH, HD = 2, 2048, 2048, 16, 128
HPC = 4              # heads per core
PAIR = 256           # queries per attention group
NPAIR = T // PAIR    # 8
NCHUNK = T // HD     # 16 key chunks of 128
PROJ = 512           # queries per projection slice
NSLICE = T // PROJ   # 4
BF16 = mybir.dt.bfloat16
F32 = mybir.dt.float32
NPBF16 = np.dtype(ml_dtypes.bfloat16)


def _mask_structure(mask):
    """Classify each (query-group, key-chunk) block of the additive mask.

    Returns (statuses, maskt): statuses[j] is a list of
    (chunk, mask_tile_index_or_minus1) for blocks that must be computed;
    maskt is the packed (128, nmask, 256) array of deduplicated
    transposed mask tiles for partially-masked blocks.
    """
    statuses = []
    tiles = {}
    tile_list = []
    for j in range(NPAIR):
        q = slice(j * PAIR, (j + 1) * PAIR)
        lst = []
        for c in range(NCHUNK):
            k = slice(c * HD, (c + 1) * HD)
            sub = mask[q, k]
            if np.all(sub <= -1e8):
                continue
            if np.all(sub == 0.0):
                lst.append((c, -1))
            else:
                key = sub.tobytes()
                mi = tiles.get(key)
                if mi is None:
                    mi = len(tile_list)
                    tiles[key] = mi
                    tile_list.append(np.ascontiguousarray(sub.T))
                lst.append((c, mi))
        assert lst, f"query group {j} has every key block masked"
        statuses.append(lst)
    nmask = max(1, len(tile_list))
    maskt = np.zeros((HD, nmask, PAIR), np.float32)
    for i, t in enumerate(tile_list):
        assert np.all(t <= 64.0), "additive mask too large for exp-mask trick"
        maskt[:, i, :] = np.exp(t)
    return statuses, maskt


def _build_program(statuses, nmask):
    nc = bacc.Bacc(None, target_bir_lowering=False)

    # All inputs are laid out on the host so that each SBUF partition's
    # content is one contiguous DRAM run (16KB descriptors).  Fine-grained
    # (1KB) descriptors cap a DMA queue at ~80GB/s because of per-descriptor
    # processing overhead; 16KB descriptors run at the full bus rate.
    xt_d = nc.dram_tensor("xts", [NSLICE, HD, NCHUNK, PROJ], BF16,
                          kind="ExternalInput")
    wq_d = nc.dram_tensor("wqt", [HD, NCHUNK, HPC * HD], BF16, kind="ExternalInput")
    wk_d = nc.dram_tensor("wkt", [HD, NCHUNK, HPC * HD], BF16, kind="ExternalInput")
    wv_d = nc.dram_tensor("wvt", [HD, NCHUNK, HPC * HD], BF16, kind="ExternalInput")
    wo_d = nc.dram_tensor("wot", [HD, HPC, D], BF16, kind="ExternalInput")
    cs_d = nc.dram_tensor("cs", [HD, 2, T], F32, kind="ExternalInput")
    mk_d = nc.dram_tensor("maskt", [HD, nmask, PAIR], BF16, kind="ExternalInput")
    ones_d = nc.dram_tensor("ones_sq", [HD, HD], BF16, kind="ExternalInput")
    out_d = nc.dram_tensor("out", [T, D], F32, kind="ExternalOutput")

    xt_ap = xt_d.ap()
    wq_ap = wq_d.ap()
    wk_ap = wk_d.ap()
    wv_ap = wv_d.ap()
    wo_ap = wo_d.ap()
    EXP = mybir.ActivationFunctionType.Exp

    with tile.TileContext(nc) as tc, ExitStack() as top:
        constp = top.enter_context(tc.tile_pool(name="const", bufs=1))
        ones_sb = constp.tile([HD, HD], BF16)  # DMA'd after wq.hi/wv below

        qkp = top.enter_context(tc.tile_pool(name="qkp", bufs=1))
        # q heads at [:, h, :], k heads at [:, 4+h, :]
        qk_sb = qkp.tile([HD, 2 * HPC, T], BF16)

        # wv + xt pools span the q/k pass (prefetch) and the v pass
        with ExitStack() as vph:
            wvp = vph.enter_context(tc.tile_pool(name="wvp", side="right", bufs=1))
            wv_sb = wvp.tile([HD, NCHUNK, HPC * HD], BF16)
            xtp = vph.enter_context(tc.tile_pool(name="xtp", side="right", bufs=2))

            # ---- combined q/k projection pass (+ fused RoPE) ----
            with ExitStack() as ph:
                wp = ph.enter_context(tc.tile_pool(name="wp", side="right", bufs=1))
                csp = ph.enter_context(tc.tile_pool(name="csp", side="right", bufs=2))
                ropep = ph.enter_context(tc.tile_pool(name="ropep", side="right", bufs=2))
                pps = ph.enter_context(tc.tile_pool(name="pps", bufs=6, space="PSUM"))
                wqk_sb = wp.tile([HD, 2, NCHUNK, HPC * HD], BF16)
                # Startup choreography.  One dma_start = one semaphore, so a
                # consumer waits for the WHOLE transfer; but each dma_start
                # also costs ~0.6us issue time on its engine.  Compromise:
                # wq moves as two halves on two queues (both land ~6us), wk
                # follows wq.lo on scalar; xt slice 0 moves as two halves so
                # the first matmuls can start on chunks 0-7.
                half = NCHUNK // 2
                nc.scalar.dma_start(wqk_sb[:, 0, 0:half, :], wq_ap[:, 0:half, :])
                nc.gpsimd.dma_start(wqk_sb[:, 0, half:, :], wq_ap[:, half:, :])
                nc.scalar.dma_start(wqk_sb[:, 1, :, :], wk_ap[:])
                nc.gpsimd.dma_start(wv_sb[:], wv_ap[:])
                nc.gpsimd.dma_start(ones_sb[:], ones_d[:])
                for ns in range(NSLICE):
                    tsl = slice(ns * PROJ, (ns + 1) * PROJ)
                    xt = xtp.tile([HD, NCHUNK, PROJ], BF16, tag="xt")
                    if ns == 0:
                        nc.sync.dma_start(xt[:, 0:half, :], xt_ap[ns, :, 0:half, :])
                        nc.sync.dma_start(xt[:, half:, :], xt_ap[ns, :, half:, :])
                    else:
                        nc.sync.dma_start(xt[:], xt_ap[ns])
                    cs_sl = csp.tile([HD, 2, PROJ], F32, tag="cs")
                    nc.sync.dma_start(cs_sl[:], cs_d[:, :, tsl])
                    for wsel in range(2):
                        for h in range(HPC):
                            ps = pps.tile([HD, PROJ], F32, tag="ps")
                            hs = slice(h * HD, (h + 1) * HD)
                            for k in range(NCHUNK):
                                nc.tensor.matmul(
                                    ps[:],
                                    wqk_sb[:, wsel, k, hs],
                                    xt[:, k, :],
                                    start=(k == 0),
                                    stop=(k == NCHUNK - 1),
                                )
                            # RoPE: dst = raw*C + swap(raw)*S.  The swap is
                            # materialized by two ScalarE half-copies; VectorE
                            # does two multiplies and one add per tile.
                            dst = qk_sb[:, wsel * HPC + h, tsl]
                            sw = ropep.tile([HD, PROJ], F32, tag="sw")
                            nc.scalar.copy(sw[0:64, :], ps[64:128, :])
                            nc.scalar.copy(sw[64:128, :], ps[0:64, :])
                            tb = ropep.tile([HD, PROJ], F32, tag="tb")
                            nc.vector.tensor_mul(dst, ps[:], cs_sl[:, 0, :])
                            nc.vector.tensor_mul(tb[:], sw[:], cs_sl[:, 1, :])
                            nc.vector.tensor_add(dst, dst, tb[:])

            # ---- v projection (normal layout, x^T stationary) ----
            vap = top.enter_context(tc.tile_pool(name="vap", bufs=1))
            v_all = vap.tile([HD, NCHUNK, HPC * HD], BF16)
            with ExitStack() as ph:
                vps = ph.enter_context(tc.tile_pool(name="vps", bufs=6, space="PSUM"))
                for ns in reversed(range(NSLICE)):
                    xt = xtp.tile([HD, NCHUNK, PROJ], BF16, tag="xt")
                    nc.sync.dma_start(xt[:], xt_ap[ns])
                    for tc2 in range(PROJ // HD):
                        ps = vps.tile([HD, HPC * HD], F32, tag="vps")
                        for k in range(NCHUNK):
                            nc.tensor.matmul(
                                ps[:],
                                xt[:, k, tc2 * HD:(tc2 + 1) * HD],
                                wv_sb[:, k, :],
                                start=(k == 0),
                                stop=(k == NCHUNK - 1),
                            )
                        nc.scalar.copy(v_all[:, ns * (PROJ // HD) + tc2, :], ps[:])

        # ---- attention + interleaved output projection ----
        ctxp = top.enter_context(tc.tile_pool(name="ctxp", bufs=1))
        ctx_sb = ctxp.tile([HD, HPC, T], BF16)
        wop = top.enter_context(tc.tile_pool(name="wop", bufs=1))
        wo_sb = wop.tile([HD, HPC, D], BF16)
        with ExitStack() as ph:
            ptp = ph.enter_context(tc.tile_pool(name="ptp", side="right", bufs=2))
            mkpre = ph.enter_context(tc.tile_pool(name="mkpre", side="right", bufs=1))
            lrp = ph.enter_context(tc.tile_pool(name="lrp", side="right", bufs=2))
            evp = ph.enter_context(tc.tile_pool(name="evp", side="right", bufs=4))
            sps = ph.enter_context(tc.tile_pool(name="sps", bufs=2, space="PSUM"))
            olps = ph.enter_context(tc.tile_pool(name="olps", bufs=2, space="PSUM"))
            wops = ph.enter_context(tc.tile_pool(name="wops", bufs=2, space="PSUM"))

            mk_sb = mkpre.tile([HD, nmask, PAIR], BF16)
            nc.gpsimd.dma_start(mk_sb[:], mk_d[:])
            nc.gpsimd.dma_start(wo_sb[:], wo_ap[:])  # prefetch wo

            def finalize(st):
                # DVE fast-recip of the (already partition-broadcast) row
                # sums, then one DVE multiply into ctx
                lr = lrp.tile([HD, PAIR], F32, tag="lr")
                nc.vector.reciprocal_approx_fast(lr[:], st["l"])
                nc.vector.tensor_mul(
                    ctx_sb[:, st["h"], st["qsl"]], st["o"], lr[:]
                )

            def emit_outproj(j):
                # output projection for query group j (all heads final)
                for tck in range(PAIR // HD):
                    tsl = slice(j * PAIR + tck * HD, j * PAIR + (tck + 1) * HD)
                    for es in range(D // 512):
                        esl = slice(es * 512, (es + 1) * 512)
                        ps = wops.tile([HD, 512], F32, tag="wo")
                        for h in range(HPC):
                            nc.tensor.matmul(
                                ps[:],
                                ctx_sb[:, h, tsl],
                                wo_sb[:, h, esl],
                                start=(h == 0),
                                stop=(h == HPC - 1),
                            )
                        ev = evp.tile([HD, 512], F32, tag="ev")
                        nc.vector.tensor_copy(ev[:], ps[:])
                        nc.sync.dma_start(out_d[tsl, esl], ev[:])

            def emit_ol(dq):
                # deferred p@v matmuls for an exp'd quad.  o and l share one
                # PSUM bank, so l's accumulation group must not open while
                # o's is still open: all l matmuls are emitted as one block
                # right after o's group closes.
                pi, quad, st = dq
                h = st["h"]
                for t, (c, mi) in enumerate(quad):
                    nc.tensor.matmul(
                        st["o"],
                        v_all[:, c, h * HD:(h + 1) * HD],
                        st["pt"][:, pi + t, :],
                        start=(st["oi"] == 0),
                        stop=(st["oi"] == st["n"] - 1),
                        skip_group_check=True,
                    )
                    st["oi"] += 1
                if st["oi"] < st["n"]:
                    return False
                for li in range(st["n"]):
                    nc.tensor.matmul(
                        st["l"],
                        ones_sb[:],
                        st["pt"][:, li, :],
                        start=(li == 0),
                        stop=(li == st["n"] - 1),
                        skip_group_check=True,
                    )
                return True

            pending_ol = None
            pending_fin = None
            for j in reversed(range(NPAIR)):
                qsl = slice(j * PAIR, (j + 1) * PAIR)
                chunks = list(reversed(statuses[j]))
                n = len(chunks)
                quads = [chunks[ii:ii + 4] for ii in range(0, n, 4)]
                for h in range(HPC):
                    ol_ps = olps.tile([HD, 2 * PAIR], F32, tag="ol")
                    pt = ptp.tile([HD, NCHUNK, PAIR], BF16, tag="pt")
                    st = {"o": ol_ps[:, 0:PAIR], "l": ol_ps[:, PAIR:2 * PAIR],
                          "pt": pt, "h": h, "qsl": qsl, "j": j, "n": n,
                          "oi": 0, "li": 0}
                    for qi, quad in enumerate(quads):
                        w = len(quad)
                        s_ps = sps.tile([HD, 4, PAIR], F32, tag="s")
                        for t, (c, mi) in enumerate(quad):
                            nc.tensor.matmul(
                                s_ps[:, t, :],
                                qk_sb[:, HPC + h, c * HD:(c + 1) * HD],
                                qk_sb[:, h, qsl],
                                start=True,
                                stop=True,
                            )
                        nc.scalar.activation(
                            pt[:, qi * 4:qi * 4 + w, :], s_ps[:, 0:w, :], EXP
                        )
                        # multiplicative exp-mask applied to pt
                        # (exp(s+m) == exp(s)*exp(m)), off the exp chain
                        t = 0
                        while t < w:
                            c, mi = quad[t]
                            if mi < 0:
                                t += 1
                                continue
                            r = t + 1
                            while (r < w and quad[r][1] >= 0
                                   and quad[r][1] == quad[r - 1][1] + 1):
                                r += 1
                            sl = slice(qi * 4 + t, qi * 4 + r)
                            nc.vector.tensor_mul(
                                pt[:, sl, :], pt[:, sl, :],
                                mk_sb[:, mi:mi + (r - t), :],
                            )
                            t = r
                        if pending_ol is not None:
                            if emit_ol(pending_ol):
                                pending_fin = pending_ol[2]
                            pending_ol = None
                        if pending_fin is not None and pending_fin is not st:
                            fs = pending_fin
                            finalize(fs)
                            pending_fin = None
                            if fs["h"] == HPC - 1:
                                emit_outproj(fs["j"])
                        pending_ol = (qi * 4, quad, st)
            if pending_ol is not None:
                if emit_ol(pending_ol):
                    pending_fin = pending_ol[2]
            if pending_fin is not None:
                finalize(pending_fin)
                if pending_fin["h"] == HPC - 1:
                    emit_outproj(pending_fin["j"])
    nc.compile()
    return nc


_PERM = np.concatenate(
    [np.concatenate([np.arange(0, HD, 2), np.arange(1, HD, 2)]) + h * HD
     for h in range(HPC)]
)


def prepare(x, freqs, mask, wq, wk, wv, wo):
    """Host-side sharding/prep. Returns (nc, in_maps)."""
    x = np.asarray(x, np.float32)
    freqs = np.asarray(freqs, np.float32)
    mask = np.asarray(mask, np.float32)
    wq, wk, wv, wo = (np.asarray(w, np.float32) for w in (wq, wk, wv, wo))

    statuses, maskt = _mask_structure(mask)
    nc = _build_program(statuses, maskt.shape[1])

    scale = np.float32(1.0 / np.sqrt(HD))
    cos = np.ascontiguousarray(freqs[:, :, 0].T)  # (64, T)
    sin = np.ascontiguousarray(freqs[:, :, 1].T)
    cs = np.empty((HD, 2, T), np.float32)
    cs[0:64, 0, :] = cos
    cs[64:128, 0, :] = cos
    cs[0:64, 1, :] = -sin
    cs[64:128, 1, :] = sin

    ones_sq = np.ones((HD, HD), NPBF16)
    maskt_bf = maskt.astype(NPBF16)

    def pshuf_w(w):
        # [D, E] -> [HD, NCHUNK, E]: partition-major so each SBUF
        # partition's content is one contiguous DRAM run
        return np.ascontiguousarray(
            w.reshape(NCHUNK, HD, w.shape[1]).transpose(1, 0, 2)
        ).astype(NPBF16)

    def pshuf_x(xb):
        # x[b].T [D, T] -> [NSLICE, HD, NCHUNK, PROJ]
        xT = xb.T.reshape(NCHUNK, HD, NSLICE, PROJ)
        return np.ascontiguousarray(xT.transpose(2, 1, 0, 3)).astype(NPBF16)

    xts = [pshuf_x(x[b]) for b in range(B)]

    in_maps = []
    for core in range(8):
        b, g = core // 4, core % 4
        cols = slice(g * HPC * HD, (g + 1) * HPC * HD)
        wot = wo.T[cols, :].reshape(HPC, HD, D).transpose(1, 0, 2)
        in_maps.append({
            "xts": xts[b],
            "wqt": pshuf_w((wq.T[:, cols] * scale)[:, _PERM]),
            "wkt": pshuf_w(wk.T[:, cols][:, _PERM]),
            "wvt": pshuf_w(wv.T[:, cols]),
            "wot": np.ascontiguousarray(wot).astype(NPBF16),
            "cs": cs,
            "maskt": maskt_bf,
            "ones_sq": ones_sq,
        })
    return nc, in_maps


def run(x, freqs, mask, wq, wk, wv, wo, **spmd_kwargs):
    nc, in_maps = prepare(x, freqs, mask, wq, wk, wv, wo)
    res = run_bass_kernel_spmd(nc, in_maps, list(range(8)), **spmd_kwargs)
    parts = [res.results[c]["out"] for c in range(8)]
    out = np.stack([
        parts[b * 4] + parts[b * 4 + 1] + parts[b * 4 + 2] + parts[b * 4 + 3]
        for b in range(B)
    ]).astype(np.float32)
    return out, res


def kernel(x, freqs, mask, wq, wk, wv, wo):
    out, _ = run(x, freqs, mask, wq, wk, wv, wo)
    return out
